# revision 5
# baseline (speedup 1.0000x reference)
"""DeltaDequantization Trainium2 kernel (8-core SPMD, pure data parallel over batch).

Math (per batch element b, chunks c of 32 steps):
    scale_c = (1/32) * sum_{s,n} x[b,c,s,n] * cs[n]          (independent of carry!)
    S_c     = prod_{c'<c} scale_c'          (exclusive cumprod)
    y[b,t]  = sum_n x[b,t,n] * qb[n]
    m_c     = (1/32) * sum_{s in c} y[b,t]
    pred_c  = sum_{c'<c} S_c' * m_c'        (exclusive cumsum)
    out[b,t]= pred_c(t) + S_c(t) * y[b,t]

Kernel: load x naturally [128b, (t,n)], PE-transpose 128x128 blocks to get
(t',n) on partitions, one [128,8] matmul computes y and w=x@cs for 4 t's at a
time, PE-transpose back to [b,t], tensor_tensor_scan for the 64-step
recurrences, affine, store.
"""

import numpy as np
from contextlib import ExitStack

import concourse.bass as bass
import concourse.bacc as bacc
import concourse.tile as tile
from concourse import mybir
from concourse.bass_utils import run_bass_kernel_spmd
from concourse.masks import make_identity

F32 = mybir.dt.float32

B, T, NB = 1024, 2048, 32
NCORES = 8
BS = B // NCORES          # 128 batch rows per core = full partition dim
ADAPT = 32
C = T // ADAPT            # 64 chunks
SPAN_T = 256              # timesteps per pipelined span
NSPAN = T // SPAN_T       # 8
SPAN_F = SPAN_T * NB      # 8192 f32 elements per partition per span

_cached_nc = None


def build_kernel():
    nc = bacc.Bacc("TRN2", target_bir_lowering=False, debug=False)

    x_ext = nc.dram_tensor("x", [BS, T * NB], F32, kind="ExternalInput")
    qb_ext = nc.dram_tensor("quant_bins", [NB, 1], F32, kind="ExternalInput")
    cs_ext = nc.dram_tensor("change_scales", [NB, 1], F32, kind="ExternalInput")
    out_ext = nc.dram_tensor("out", [BS, T], F32, kind="ExternalOutput")

    with tile.TileContext(nc) as tc:
        with (
            tc.tile_pool(name="consts", bufs=1) as consts,
            tc.tile_pool(name="xpool", bufs=2) as xpool,
            tc.tile_pool(name="xtpool", bufs=2) as xtpool,
            tc.tile_pool(name="midpool", bufs=2) as midpool,
            tc.tile_pool(name="accpool", bufs=1) as accpool,
            tc.tile_pool(name="smallpool", bufs=1) as smallpool,
            tc.tile_pool(name="ps_t", bufs=3, space="PSUM") as ps_t,
            tc.tile_pool(name="ps_yw", bufs=2, space="PSUM") as ps_yw,
            tc.tile_pool(name="ps_b", bufs=2, space="PSUM") as ps_b,
        ):
            ident = consts.tile([128, 128], F32)
            make_identity(nc, ident[:])

            # Four stationary matrices A32_q [128, 32], q = 0..3.
            # Column m = 16*j + 4*q + t''; A32_q[(t', n), m] = delta(t', t'') *
            # (qb[n] if j == 0 else cs[n]); zero columns for other q values.
            # Four accumulating matmuls (q = 0..3) over one 32-partition PSUM
            # strip then yield a dense (j, q, t'') x (blk, b) layout.
            A32 = []
            for q in range(4):
                Aq = consts.tile([128, 32], F32, tag=f"A32_{q}")
                nc.gpsimd.memset(Aq[:], 0.0)
                for tp in range(4):
                    nc.sync.dma_start(
                        out=Aq[32 * tp:32 * tp + 32, 4 * q + tp:4 * q + tp + 1],
                        in_=qb_ext[:],
                    )
                    nc.sync.dma_start(
                        out=Aq[32 * tp:32 * tp + 32, 16 + 4 * q + tp:16 + 4 * q + tp + 1],
                        in_=cs_ext[:],
                    )
                A32.append(Aq)

            # Persistent per-core accumulators
            y_sb = accpool.tile([128, T], F32)
            w_sb = accpool.tile([128, T], F32)
            out_sb = accpool.tile([128, T], F32)

            for sp in range(NSPAN):
                x_sp = xpool.tile([128, SPAN_F], F32)
                nc.sync.dma_start(out=x_sp[:], in_=x_ext[:, sp * SPAN_F:(sp + 1) * SPAN_F])

                xT_sp = xtpool.tile([128, SPAN_F], F32)
                # 64 transposed 128x128 blocks; 4 per PSUM bank
                for pb in range(16):
                    pst = ps_t.tile([128, 512], F32)
                    for k in range(4):
                        blk = pb * 4 + k
                        nc.tensor.transpose(
                            pst[:, k * 128:(k + 1) * 128],
                            x_sp[:, blk * 128:(blk + 1) * 128],
                            ident[:],
                        )
                    dst = xT_sp[:, pb * 512:(pb + 1) * 512]
                    if pb % 2 == 0:
                        nc.vector.tensor_copy(out=dst, in_=pst[:])
                    else:
                        nc.scalar.copy(out=dst, in_=pst[:])

                # y/w projection. Group g = g4*4 + q covers blocks 4g..4g+3
                # (t = 16g + 4blk + t''). Strip g4 accumulates 4 matmuls into
                # ps_y[32*g4 : 32*g4+32, :]; dense partition layout
                # p = 32*g4 + 16*j + 4*q + t'', free = (blk, b).
                ps_y = ps_yw.tile([128, 512], F32)
                for g4 in range(4):
                    for q in range(4):
                        g = g4 * 4 + q
                        nc.tensor.matmul(
                            ps_y[32 * g4:32 * g4 + 32, :],
                            A32[q][:],
                            xT_sp[:, g * 512:(g + 1) * 512],
                            start=(q == 0),
                            stop=(q == 3),
                            tile_position=(0, 32 * g4),
                        )

                yw_mid = midpool.tile([128, 512], F32)
                nc.vector.tensor_copy(out=yw_mid[:], in_=ps_y[:])

                ps2 = ps_b.tile([128, 512], F32)
                for blk2 in range(4):
                    nc.tensor.transpose(
                        ps2[:, blk2 * 128:(blk2 + 1) * 128],
                        yw_mid[:, blk2 * 128:(blk2 + 1) * 128],
                        ident[:],
                    )

                # ps2 free index = 128*blk + 32*g4 + 16*j + 4*q + t''
                # t(within span) = 64*g4 + 16*q + 4*blk + t''
                ps2v = ps2[:].rearrange(
                    "p (b g j q t) -> p g q b j t", b=4, g=4, j=2, q=4, t=4
                )
                yspan = y_sb[:, sp * SPAN_T:(sp + 1) * SPAN_T].rearrange(
                    "p (g q b t) -> p g q b t", g=4, q=4, b=4, t=4
                )
                wspan = w_sb[:, sp * SPAN_T:(sp + 1) * SPAN_T].rearrange(
                    "p (g q b t) -> p g q b t", g=4, q=4, b=4, t=4
                )
                nc.vector.tensor_copy(out=yspan, in_=ps2v[:, :, :, :, 0:1, :].squeeze(4))
                nc.vector.tensor_copy(out=wspan, in_=ps2v[:, :, :, :, 1:2, :].squeeze(4))

            # ---- sequential part: chunk stats, scans, affine ----
            m_term = smallpool.tile([128, C], F32)
            p_sc = smallpool.tile([128, C], F32)
            nc.vector.tensor_reduce(
                out=m_term[:],
                in_=y_sb[:].rearrange("p (c s) -> p c s", c=C, s=ADAPT),
                axis=mybir.AxisListType.X,
                op=mybir.AluOpType.add,
            )
            nc.vector.tensor_reduce(
                out=p_sc[:],
                in_=w_sb[:].rearrange("p (c s) -> p c s", c=C, s=ADAPT),
                axis=mybir.AxisListType.X,
                op=mybir.AluOpType.add,
            )
            nc.vector.tensor_scalar_mul(m_term[:], m_term[:], 1.0 / ADAPT)
            nc.vector.tensor_scalar_mul(p_sc[:], p_sc[:], 1.0 / ADAPT)

            S_sb = smallpool.tile([128, C], F32)
            pred = smallpool.tile([128, C], F32)
            tau = smallpool.tile([128, C], F32)
            nc.gpsimd.memset(S_sb[:, 0:1], 1.0)
            nc.vector.tensor_tensor_scan(
                out=S_sb[:, 1:C],
                data0=p_sc[:, 0:C - 1],
                data1=p_sc[:, 0:C - 1],
                initial=1.0,
                op0=mybir.AluOpType.mult,
                op1=mybir.AluOpType.bypass,
            )
            nc.vector.tensor_mul(out=tau[:], in0=S_sb[:], in1=m_term[:])
            nc.gpsimd.memset(pred[:, 0:1], 0.0)
            nc.vector.tensor_tensor_scan(
                out=pred[:, 1:C],
                data0=tau[:, 0:C - 1],
                data1=tau[:, 0:C - 1],
                initial=0.0,
                op0=mybir.AluOpType.add,
                op1=mybir.AluOpType.bypass,
            )

            y3 = y_sb[:].rearrange("p (c s) -> p c s", c=C, s=ADAPT)
            o3 = out_sb[:].rearrange("p (c s) -> p c s", c=C, s=ADAPT)
            S_b = S_sb[:].unsqueeze(2).broadcast_to([128, C, ADAPT])
            pred_b = pred[:].unsqueeze(2).broadcast_to([128, C, ADAPT])
            nc.vector.tensor_mul(out=o3, in0=y3, in1=S_b)
            nc.vector.tensor_add(out=o3, in0=o3, in1=pred_b)

            nc.sync.dma_start(out=out_ext[:], in_=out_sb[:])

    nc.compile()
    return nc


def kernel(x, quant_bins, change_scales):
    global _cached_nc
    if _cached_nc is None:
        _cached_nc = build_kernel()
    nc = _cached_nc

    x = np.ascontiguousarray(x, dtype=np.float32)
    qb = np.ascontiguousarray(quant_bins, dtype=np.float32).reshape(NB, 1)
    cs = np.ascontiguousarray(change_scales, dtype=np.float32).reshape(NB, 1)

    in_maps = [
        {
            "x": x[i * BS:(i + 1) * BS].reshape(BS, T * NB),
            "quant_bins": qb,
            "change_scales": cs,
        }
        for i in range(NCORES)
    ]
    res = run_bass_kernel_spmd(nc, in_maps, core_ids=list(range(NCORES)))
    out = np.concatenate([res.results[i]["out"] for i in range(NCORES)], axis=0)
    return out.astype(np.float32)


if __name__ == "__main__":
    rng = np.random.default_rng(0)
    x = rng.standard_normal((B, T, NB)).astype(np.float32)
    qb = rng.standard_normal((NB,)).astype(np.float32)
    cs = rng.uniform(0.9, 1.1, (NB, 1)).astype(np.float32)
    out = kernel(x=x, quant_bins=qb, change_scales=cs)
    print("out", out.shape, out.dtype)


# revision 8
# speedup vs baseline: 1.0577x; 1.0577x over previous
"""DeltaDequantization Trainium2 kernel (8-core SPMD, pure data parallel over batch).

Math (per batch element b, chunks c of 32 steps):
    scale_c = (1/32) * sum_{s,n} x[b,c,s,n] * cs[n]          (independent of carry!)
    S_c     = prod_{c'<c} scale_c'          (exclusive cumprod)
    y[b,t]  = sum_n x[b,t,n] * qb[n]
    m_c     = (1/32) * sum_{s in c} y[b,t]
    pred_c  = sum_{c'<c} S_c' * m_c'        (exclusive cumsum)
    out[b,t]= pred_c(t) + S_c(t) * y[b,t]

Kernel: load x naturally [128b, (t,n)], PE-transpose 128x128 blocks to get
(t',n) on partitions, one [128,8] matmul computes y and w=x@cs for 4 t's at a
time, PE-transpose back to [b,t], tensor_tensor_scan for the 64-step
recurrences, affine, store.
"""

import numpy as np
from contextlib import ExitStack

import concourse.bass as bass
import concourse.bacc as bacc
import concourse.tile as tile
from concourse import mybir
from concourse.bass_utils import run_bass_kernel_spmd
from concourse.masks import make_identity

F32 = mybir.dt.float32
BF16 = mybir.dt.bfloat16

B, T, NB = 1024, 2048, 32
NCORES = 8
BS = B // NCORES          # 128 batch rows per core = full partition dim
ADAPT = 32
C = T // ADAPT            # 64 chunks
SPAN_T = 256              # timesteps per pipelined span
NSPAN = T // SPAN_T       # 8
SPAN_F = SPAN_T * NB      # 8192 f32 elements per partition per span

_cached_nc = None


def build_kernel():
    nc = bacc.Bacc("TRN2", target_bir_lowering=False, debug=False)

    x_ext = nc.dram_tensor("x", [BS, T * NB], F32, kind="ExternalInput")
    qb_ext = nc.dram_tensor("quant_bins", [NB, 1], F32, kind="ExternalInput")
    cs_ext = nc.dram_tensor("change_scales", [NB, 1], F32, kind="ExternalInput")
    out_ext = nc.dram_tensor("out", [BS, T], F32, kind="ExternalOutput")

    with tile.TileContext(nc) as tc:
        with (
            tc.tile_pool(name="consts", bufs=1) as consts,
            tc.tile_pool(name="xpool", bufs=2) as xpool,
            tc.tile_pool(name="xtpool", bufs=2) as xtpool,
            tc.tile_pool(name="midpool", bufs=2) as midpool,
            tc.tile_pool(name="accpool", bufs=1) as accpool,
            tc.tile_pool(name="smallpool", bufs=1) as smallpool,
            tc.tile_pool(name="ps_t", bufs=3, space="PSUM") as ps_t,
            tc.tile_pool(name="ps_yw", bufs=2, space="PSUM") as ps_yw,
            tc.tile_pool(name="ps_b", bufs=2, space="PSUM") as ps_b,
        ):
            ident = consts.tile([128, 128], F32)
            make_identity(nc, ident[:])

            # Four stationary matrices A32_q [128, 32], q = 0..3.
            # Column m = 16*j + 4*q + t''; A32_q[(t', n), m] = delta(t', t'') *
            # (qb[n] if j == 0 else cs[n]); zero columns for other q values.
            # Four accumulating matmuls (q = 0..3) over one 32-partition PSUM
            # strip then yield a dense (j, q, t'') x (blk, b) layout.
            A32 = []
            for q in range(4):
                Aq = consts.tile([128, 32], BF16, tag=f"A32_{q}")
                nc.gpsimd.memset(Aq[:], 0.0)
                for tp in range(4):
                    nc.gpsimd.dma_start(
                        out=Aq[32 * tp:32 * tp + 32, 4 * q + tp:4 * q + tp + 1],
                        in_=qb_ext[:],
                    )
                    nc.gpsimd.dma_start(
                        out=Aq[32 * tp:32 * tp + 32, 16 + 4 * q + tp:16 + 4 * q + tp + 1],
                        in_=cs_ext[:],
                    )
                A32.append(Aq)

            # Persistent per-core accumulators
            y_sb = accpool.tile([128, T], F32)
            w_sb = accpool.tile([128, T], F32)
            out_sb = accpool.tile([128, T], F32)

            for sp in range(NSPAN):
                x_sp = xpool.tile([128, SPAN_F], F32)
                nc.sync.dma_start(out=x_sp[:], in_=x_ext[:, sp * SPAN_F:(sp + 1) * SPAN_F])

                xT_sp = xtpool.tile([128, SPAN_F], BF16)
                # y/w projection, interleaved with the transposes that feed it
                # so real matmuls keep the PE HAM clock warm. Group g = g4*4+q
                # covers blocks 4g..4g+3 (t = 16g + 4blk + t''). Strip g4
                # accumulates 4 matmuls into ps_y[32*g4 : 32*g4+32, :]; dense
                # partition layout p = 32*g4 + 16*j + 4*q + t'', free=(blk,b).
                ps_y = ps_yw.tile([128, 512], F32)
                for g in range(16):
                    g4, q = divmod(g, 4)
                    pst = ps_t.tile([128, 512], F32)
                    for k in range(4):
                        blk = g * 4 + k
                        nc.tensor.transpose(
                            pst[:, k * 128:(k + 1) * 128],
                            x_sp[:, blk * 128:(blk + 1) * 128],
                            ident[:],
                        )
                    dst = xT_sp[:, g * 512:(g + 1) * 512]
                    if g % 2 == 0:
                        nc.vector.tensor_copy(out=dst, in_=pst[:])
                    else:
                        nc.scalar.copy(out=dst, in_=pst[:])
                    nc.tensor.matmul(
                        ps_y[32 * g4:32 * g4 + 32, :],
                        A32[q][:],
                        dst,
                        start=(q == 0),
                        stop=(q == 3),
                        tile_position=(0, 32 * g4),
                    )

                yw_mid = midpool.tile([128, 512], F32)
                nc.vector.tensor_copy(out=yw_mid[:], in_=ps_y[:])

                ps2 = ps_b.tile([128, 512], F32)
                for blk2 in range(4):
                    nc.tensor.transpose(
                        ps2[:, blk2 * 128:(blk2 + 1) * 128],
                        yw_mid[:, blk2 * 128:(blk2 + 1) * 128],
                        ident[:],
                    )

                # ps2 free index = 128*blk + 32*g4 + 16*j + 4*q + t''
                # t(within span) = 64*g4 + 16*q + 4*blk + t''
                ps2v = ps2[:].rearrange(
                    "p (b g j q t) -> p g q b j t", b=4, g=4, j=2, q=4, t=4
                )
                yspan = y_sb[:, sp * SPAN_T:(sp + 1) * SPAN_T].rearrange(
                    "p (g q b t) -> p g q b t", g=4, q=4, b=4, t=4
                )
                wspan = w_sb[:, sp * SPAN_T:(sp + 1) * SPAN_T].rearrange(
                    "p (g q b t) -> p g q b t", g=4, q=4, b=4, t=4
                )
                nc.vector.tensor_copy(out=yspan, in_=ps2v[:, :, :, :, 0:1, :].squeeze(4))
                nc.vector.tensor_copy(out=wspan, in_=ps2v[:, :, :, :, 1:2, :].squeeze(4))

            # ---- sequential part: chunk stats, scans, affine ----
            m_term = smallpool.tile([128, C], F32)
            p_sc = smallpool.tile([128, C], F32)
            nc.vector.tensor_reduce(
                out=m_term[:],
                in_=y_sb[:].rearrange("p (c s) -> p c s", c=C, s=ADAPT),
                axis=mybir.AxisListType.X,
                op=mybir.AluOpType.add,
            )
            nc.vector.tensor_reduce(
                out=p_sc[:],
                in_=w_sb[:].rearrange("p (c s) -> p c s", c=C, s=ADAPT),
                axis=mybir.AxisListType.X,
                op=mybir.AluOpType.add,
            )
            nc.vector.tensor_scalar_mul(m_term[:], m_term[:], 1.0 / ADAPT)
            nc.vector.tensor_scalar_mul(p_sc[:], p_sc[:], 1.0 / ADAPT)

            S_sb = smallpool.tile([128, C], F32)
            pred = smallpool.tile([128, C], F32)
            tau = smallpool.tile([128, C], F32)
            nc.gpsimd.memset(S_sb[:, 0:1], 1.0)
            nc.vector.tensor_tensor_scan(
                out=S_sb[:, 1:C],
                data0=p_sc[:, 0:C - 1],
                data1=p_sc[:, 0:C - 1],
                initial=1.0,
                op0=mybir.AluOpType.mult,
                op1=mybir.AluOpType.bypass,
            )
            nc.vector.tensor_mul(out=tau[:], in0=S_sb[:], in1=m_term[:])
            nc.gpsimd.memset(pred[:, 0:1], 0.0)
            nc.vector.tensor_tensor_scan(
                out=pred[:, 1:C],
                data0=tau[:, 0:C - 1],
                data1=tau[:, 0:C - 1],
                initial=0.0,
                op0=mybir.AluOpType.add,
                op1=mybir.AluOpType.bypass,
            )

            y3 = y_sb[:].rearrange("p (c s) -> p c s", c=C, s=ADAPT)
            o3 = out_sb[:].rearrange("p (c s) -> p c s", c=C, s=ADAPT)
            S_b = S_sb[:].unsqueeze(2).broadcast_to([128, C, ADAPT])
            pred_b = pred[:].unsqueeze(2).broadcast_to([128, C, ADAPT])
            nc.vector.tensor_mul(out=o3, in0=y3, in1=S_b)
            nc.vector.tensor_add(out=o3, in0=o3, in1=pred_b)

            nc.sync.dma_start(out=out_ext[:], in_=out_sb[:])

    nc.compile()
    return nc


def kernel(x, quant_bins, change_scales):
    global _cached_nc
    if _cached_nc is None:
        _cached_nc = build_kernel()
    nc = _cached_nc

    x = np.ascontiguousarray(x, dtype=np.float32)
    qb = np.ascontiguousarray(quant_bins, dtype=np.float32).reshape(NB, 1)
    cs = np.ascontiguousarray(change_scales, dtype=np.float32).reshape(NB, 1)

    in_maps = [
        {
            "x": x[i * BS:(i + 1) * BS].reshape(BS, T * NB),
            "quant_bins": qb,
            "change_scales": cs,
        }
        for i in range(NCORES)
    ]
    res = run_bass_kernel_spmd(nc, in_maps, core_ids=list(range(NCORES)))
    out = np.concatenate([res.results[i]["out"] for i in range(NCORES)], axis=0)
    return out.astype(np.float32)


if __name__ == "__main__":
    rng = np.random.default_rng(0)
    x = rng.standard_normal((B, T, NB)).astype(np.float32)
    qb = rng.standard_normal((NB,)).astype(np.float32)
    cs = rng.uniform(0.9, 1.1, (NB, 1)).astype(np.float32)
    out = kernel(x=x, quant_bins=qb, change_scales=cs)
    print("out", out.shape, out.dtype)


# revision 10
# speedup vs baseline: 1.2452x; 1.1773x over previous
"""DeltaDequantization Trainium2 kernel (8-core SPMD, pure data parallel over batch).

Math (per batch element b, chunks c of 32 steps):
    scale_c = (1/32) * sum_{s,n} x[b,c,s,n] * cs[n]          (independent of carry!)
    S_c     = prod_{c'<c} scale_c'          (exclusive cumprod)
    y[b,t]  = sum_n x[b,t,n] * qb[n]
    m_c     = (1/32) * sum_{s in c} y[b,t]
    pred_c  = sum_{c'<c} S_c' * m_c'        (exclusive cumsum)
    out[b,t]= pred_c(t) + S_c(t) * y[b,t]

Kernel: load x naturally [128b, (t,n)], PE-transpose 128x128 blocks to get
(t',n) on partitions, one [128,8] matmul computes y and w=x@cs for 4 t's at a
time, PE-transpose back to [b,t], tensor_tensor_scan for the 64-step
recurrences, affine, store.
"""

import numpy as np
from contextlib import ExitStack

import concourse.bass as bass
import concourse.bacc as bacc
import concourse.tile as tile
from concourse import mybir
from concourse.bass_utils import run_bass_kernel_spmd
from concourse.masks import make_identity

F32 = mybir.dt.float32
BF16 = mybir.dt.bfloat16

B, T, NB = 1024, 2048, 32
NCORES = 8
BS = B // NCORES          # 128 batch rows per core = full partition dim
ADAPT = 32
C = T // ADAPT            # 64 chunks
SPAN_T = 256              # timesteps per pipelined span
NSPAN = T // SPAN_T       # 8
SPAN_F = SPAN_T * NB      # 8192 f32 elements per partition per span

_cached_nc = None


def build_kernel():
    nc = bacc.Bacc("TRN2", target_bir_lowering=False, debug=False)

    x_ext = nc.dram_tensor("x", [BS, T * NB], F32, kind="ExternalInput")
    qb_ext = nc.dram_tensor("quant_bins", [NB, 1], F32, kind="ExternalInput")
    cs_ext = nc.dram_tensor("change_scales", [NB, 1], F32, kind="ExternalInput")
    out_ext = nc.dram_tensor("out", [BS, T], F32, kind="ExternalOutput")

    with tile.TileContext(nc) as tc:
        with (
            tc.tile_pool(name="consts", bufs=1) as consts,
            tc.tile_pool(name="xpool", bufs=2) as xpool,
            tc.tile_pool(name="xtpool", bufs=2) as xtpool,
            tc.tile_pool(name="midpool", bufs=2) as midpool,
            tc.tile_pool(name="accpool", bufs=1) as accpool,
            tc.tile_pool(name="smallpool", bufs=1) as smallpool,
            tc.tile_pool(name="ps_t", bufs=3, space="PSUM") as ps_t,
            tc.tile_pool(name="ps_yw", bufs=2, space="PSUM") as ps_yw,
            tc.tile_pool(name="ps_b", bufs=2, space="PSUM") as ps_b,
        ):
            ident = consts.tile([128, 128], F32)
            make_identity(nc, ident[:])
            ident_bf = consts.tile([128, 128], BF16)
            make_identity(nc, ident_bf[:])

            # Four stationary matrices A32_q [128, 32], q = 0..3.
            # Column m = 16*j + 4*q + t''; A32_q[(t', n), m] = delta(t', t'') *
            # (qb[n] if j == 0 else cs[n]); zero columns for other q values.
            # Four accumulating matmuls (q = 0..3) over one 32-partition PSUM
            # strip then yield a dense (j, q, t'') x (blk, b) layout.
            A32 = []
            for q in range(4):
                Aq = consts.tile([128, 32], BF16, tag=f"A32_{q}")
                nc.gpsimd.memset(Aq[:], 0.0)
                for tp in range(4):
                    nc.gpsimd.dma_start(
                        out=Aq[32 * tp:32 * tp + 32, 4 * q + tp:4 * q + tp + 1],
                        in_=qb_ext[:],
                    )
                    nc.gpsimd.dma_start(
                        out=Aq[32 * tp:32 * tp + 32, 16 + 4 * q + tp:16 + 4 * q + tp + 1],
                        in_=cs_ext[:],
                    )
                A32.append(Aq)

            # Persistent per-core accumulators
            y_sb = accpool.tile([128, T], F32)
            w_sb = accpool.tile([128, T], F32)
            out_sb = accpool.tile([128, T], F32)

            for sp in range(NSPAN):
                # SWDGE cast-load: f32 DRAM -> bf16 SBUF at HBM line rate
                x_sp = xpool.tile([128, SPAN_F], BF16)
                nc.gpsimd.dma_start(out=x_sp[:], in_=x_ext[:, sp * SPAN_F:(sp + 1) * SPAN_F])

                xT_sp = xtpool.tile([128, SPAN_F], BF16)
                # y/w projection, interleaved with the transposes that feed it
                # so real matmuls keep the PE HAM clock warm. Group g = g4*4+q
                # covers blocks 4g..4g+3 (t = 16g + 4blk + t''). Strip g4
                # accumulates 4 matmuls into ps_y[32*g4 : 32*g4+32, :]; dense
                # partition layout p = 32*g4 + 16*j + 4*q + t'', free=(blk,b).
                ps_y = ps_yw.tile([128, 512], F32)
                for pb in range(8):
                    pst = ps_t.tile([128, 1024], BF16)
                    for k in range(8):
                        blk = pb * 8 + k
                        nc.tensor.transpose(
                            pst[:, k * 128:(k + 1) * 128],
                            x_sp[:, blk * 128:(blk + 1) * 128],
                            ident_bf[:],
                        )
                    dst = xT_sp[:, pb * 1024:(pb + 1) * 1024]
                    if pb % 2 == 0:
                        nc.vector.tensor_copy(out=dst, in_=pst[:])
                    else:
                        nc.scalar.copy(out=dst, in_=pst[:])
                    for gg in range(2):
                        g = pb * 2 + gg
                        g4, q = divmod(g, 4)
                        nc.tensor.matmul(
                            ps_y[32 * g4:32 * g4 + 32, :],
                            A32[q][:],
                            xT_sp[:, g * 512:(g + 1) * 512],
                            start=(q == 0),
                            stop=(q == 3),
                            tile_position=(0, 32 * g4),
                        )

                yw_mid = midpool.tile([128, 512], F32)
                nc.vector.tensor_copy(out=yw_mid[:], in_=ps_y[:])

                ps2 = ps_b.tile([128, 512], F32)
                for blk2 in range(4):
                    nc.tensor.transpose(
                        ps2[:, blk2 * 128:(blk2 + 1) * 128],
                        yw_mid[:, blk2 * 128:(blk2 + 1) * 128],
                        ident[:],
                    )

                # ps2 free index = 128*blk + 32*g4 + 16*j + 4*q + t''
                # t(within span) = 64*g4 + 16*q + 4*blk + t''
                ps2v = ps2[:].rearrange(
                    "p (b g j q t) -> p g q b j t", b=4, g=4, j=2, q=4, t=4
                )
                yspan = y_sb[:, sp * SPAN_T:(sp + 1) * SPAN_T].rearrange(
                    "p (g q b t) -> p g q b t", g=4, q=4, b=4, t=4
                )
                wspan = w_sb[:, sp * SPAN_T:(sp + 1) * SPAN_T].rearrange(
                    "p (g q b t) -> p g q b t", g=4, q=4, b=4, t=4
                )
                nc.vector.tensor_copy(out=yspan, in_=ps2v[:, :, :, :, 0:1, :].squeeze(4))
                nc.vector.tensor_copy(out=wspan, in_=ps2v[:, :, :, :, 1:2, :].squeeze(4))

            # ---- sequential part: chunk stats, scans, affine ----
            m_term = smallpool.tile([128, C], F32)
            p_sc = smallpool.tile([128, C], F32)
            nc.vector.tensor_reduce(
                out=m_term[:],
                in_=y_sb[:].rearrange("p (c s) -> p c s", c=C, s=ADAPT),
                axis=mybir.AxisListType.X,
                op=mybir.AluOpType.add,
            )
            nc.vector.tensor_reduce(
                out=p_sc[:],
                in_=w_sb[:].rearrange("p (c s) -> p c s", c=C, s=ADAPT),
                axis=mybir.AxisListType.X,
                op=mybir.AluOpType.add,
            )
            nc.vector.tensor_scalar_mul(m_term[:], m_term[:], 1.0 / ADAPT)
            nc.vector.tensor_scalar_mul(p_sc[:], p_sc[:], 1.0 / ADAPT)

            S_sb = smallpool.tile([128, C], F32)
            pred = smallpool.tile([128, C], F32)
            tau = smallpool.tile([128, C], F32)
            nc.gpsimd.memset(S_sb[:, 0:1], 1.0)
            nc.vector.tensor_tensor_scan(
                out=S_sb[:, 1:C],
                data0=p_sc[:, 0:C - 1],
                data1=p_sc[:, 0:C - 1],
                initial=1.0,
                op0=mybir.AluOpType.mult,
                op1=mybir.AluOpType.bypass,
            )
            nc.vector.tensor_mul(out=tau[:], in0=S_sb[:], in1=m_term[:])
            nc.gpsimd.memset(pred[:, 0:1], 0.0)
            nc.vector.tensor_tensor_scan(
                out=pred[:, 1:C],
                data0=tau[:, 0:C - 1],
                data1=tau[:, 0:C - 1],
                initial=0.0,
                op0=mybir.AluOpType.add,
                op1=mybir.AluOpType.bypass,
            )

            y3 = y_sb[:].rearrange("p (c s) -> p c s", c=C, s=ADAPT)
            o3 = out_sb[:].rearrange("p (c s) -> p c s", c=C, s=ADAPT)
            S_b = S_sb[:].unsqueeze(2).broadcast_to([128, C, ADAPT])
            pred_b = pred[:].unsqueeze(2).broadcast_to([128, C, ADAPT])
            nc.vector.tensor_mul(out=o3, in0=y3, in1=S_b)
            nc.vector.tensor_add(out=o3, in0=o3, in1=pred_b)

            nc.sync.dma_start(out=out_ext[:], in_=out_sb[:])

    nc.compile()
    return nc


def kernel(x, quant_bins, change_scales):
    global _cached_nc
    if _cached_nc is None:
        _cached_nc = build_kernel()
    nc = _cached_nc

    x = np.ascontiguousarray(x, dtype=np.float32)
    qb = np.ascontiguousarray(quant_bins, dtype=np.float32).reshape(NB, 1)
    cs = np.ascontiguousarray(change_scales, dtype=np.float32).reshape(NB, 1)

    in_maps = [
        {
            "x": x[i * BS:(i + 1) * BS].reshape(BS, T * NB),
            "quant_bins": qb,
            "change_scales": cs,
        }
        for i in range(NCORES)
    ]
    res = run_bass_kernel_spmd(nc, in_maps, core_ids=list(range(NCORES)))
    out = np.concatenate([res.results[i]["out"] for i in range(NCORES)], axis=0)
    return out.astype(np.float32)


if __name__ == "__main__":
    rng = np.random.default_rng(0)
    x = rng.standard_normal((B, T, NB)).astype(np.float32)
    qb = rng.standard_normal((NB,)).astype(np.float32)
    cs = rng.uniform(0.9, 1.1, (NB, 1)).astype(np.float32)
    out = kernel(x=x, quant_bins=qb, change_scales=cs)
    print("out", out.shape, out.dtype)
